# revision 39
# baseline (speedup 1.0000x reference)
import sys

for _p in ("/opt/trn_rl_repo",):
    if _p not in sys.path:
        sys.path.insert(0, _p)

import numpy as np

B, G, DIM, N = 4, 512, 384, 25088
IMAGE = 224
KS = 8
POOL = IMAGE // KS            # 28
NCORES = 8
HALF = N // 2                 # 12544 points per core
PPT = 128                     # points per tile
NT = HALF // PPT              # 98 normal tiles
CELLS = 7 * POOL              # 196 pool cells per core
TB = 20                       # tiles per weight-math batch
GRIDB = 32                    # host pruning grid (32^3)
WIDE_PT = 128                 # point cand-range width beyond which -> wide tile
SLAB = 256                    # max normal window width

_CACHE = {}


# ---------------------------------------------------------------- host: plan

def _morton_order(cen, bits=3):
    g = np.clip((cen * (1 << bits)).astype(np.int64), 0, (1 << bits) - 1)
    h = np.zeros(cen.shape[0], dtype=np.int64)
    for b in range(bits - 1, -1, -1):
        for i in range(3):
            h = (h << 1) | ((g[:, i] >> b) & 1)
    return np.argsort(h, kind="stable")


def _cell_r3(cen_s):
    """Per grid-cell conservative 3NN radius bound: r3(cellcenter)+halfdiag."""
    ncell = GRIDB
    cs = 1.0 / ncell
    halfdiag = cs * np.sqrt(3.0) / 2.0
    ax = (np.arange(ncell) + 0.5) * cs
    cc = np.stack(np.meshgrid(ax, ax, ax, indexing="ij"), axis=-1).reshape(-1, 3)
    c32 = cen_s.astype(np.float32)
    cc32 = cc.astype(np.float32)
    d2 = (
        (cc32 ** 2).sum(1)[:, None]
        + (c32 ** 2).sum(1)[None, :]
        - 2.0 * cc32 @ c32.T
    )
    d = np.sqrt(np.maximum(d2, 0.0))
    r3 = np.partition(d, 2, axis=1)[:, 2]
    return (r3 + halfdiag).astype(np.float64)


def _point_ranges(pts, cen_s, r3cell):
    """Per-point conservative candidate gid range [pa, pb] (inclusive)."""
    ci = (
        np.clip((pts * GRIDB).astype(np.int64), 0, GRIDB - 1)
        @ np.array([GRIDB * GRIDB, GRIDB, 1])
    )
    rad = r3cell[ci] + 1e-6                      # d3(p) upper bound
    p32 = pts.astype(np.float32)
    c32 = cen_s.astype(np.float32)
    d2 = (
        (p32 ** 2).sum(1)[:, None]
        + (c32 ** 2).sum(1)[None, :]
        - 2.0 * p32 @ c32.T
    )
    mask = d2 <= (rad ** 2)[:, None].astype(np.float32)
    gids = np.arange(G)
    pa = np.where(mask, gids[None, :], G).min(axis=1)
    pb = np.where(mask, gids[None, :], -1).max(axis=1)
    return pa, pb


def _plan(group_centers, original_points):
    """Build the shared SPMD tile plan + per-core assignments."""
    centers = np.asarray(group_centers, dtype=np.float64)
    points = np.asarray(original_points, dtype=np.float64)

    batch = []
    for b in range(B):
        order = _morton_order(centers[b])
        cen_s = centers[b][order]
        r3cell = _cell_r3(cen_s)
        batch.append((order, cen_s, r3cell))

    cores = []
    max_wide = 0
    for c in range(NCORES):
        b, h = c // 2, c % 2
        order, cen_s, r3cell = batch[b]
        pts = points[b, h * HALF:(h + 1) * HALF]
        pa, pb = _point_ranges(pts, cen_s, r3cell)
        width = pb - pa + 1
        wide = width > WIDE_PT
        max_wide = max(max_wide, int(wide.sum()))
        nrm = np.nonzero(~wide)[0]
        mid = (pa[nrm] + pb[nrm]).astype(np.float64) * 0.5
        nrm = nrm[np.argsort(mid, kind="stable")]
        cores.append({"pa": pa, "pb": pb, "nrm": nrm, "wide": np.nonzero(wide)[0]})

    n_wide_tiles = (max_wide + PPT - 1) // PPT
    T = NT + n_wide_tiles

    for core in cores:
        src = np.full((T, PPT), -1, dtype=np.int64)
        nrm = core["nrm"]
        per = len(nrm) // NT
        extra = len(nrm) - per * NT
        pos = 0
        for t in range(NT):
            k = per + (1 if t < extra else 0)
            src[t, :k] = nrm[pos:pos + k]
            pos += k
        w = core["wide"]
        for j, t in enumerate(range(NT, T)):
            seg = w[j * PPT:(j + 1) * PPT]
            src[t, :len(seg)] = seg
        core["src"] = src

    # program windows: union over cores, 32-aligned; wide tiles scan all
    lo = np.zeros(T, dtype=np.int64)
    hi = np.full(T, G, dtype=np.int64)
    for t in range(NT):
        amin, bmax = G, -1
        for core in cores:
            s = core["src"][t]
            s = s[s >= 0]
            if len(s):
                amin = min(amin, core["pa"][s].min())
                bmax = max(bmax, core["pb"][s].max())
        if bmax < 0:
            amin, bmax = 0, 31
        l = (amin // 32) * 32
        hh = min(G, ((bmax + 1 + 31) // 32) * 32)
        if hh - l > SLAB:
            l, hh = 0, G
        lo[t], hi[t] = l, hh

    return {"T": T, "lo": lo, "hi": hi, "batch": batch, "cores": cores}


def _legal_pieces(lo, hi):
    """Split window [lo,hi) into matmul-legal (group64, ppos, m, wtoff) pieces.

    W^T accumulates in 8 psum tiles of (64, CELLS); legal out partition
    offsets within a tile are 0 (m<=64) and 32 (m<=32)."""
    pieces = []
    x = lo
    while x < hi:
        g = x // 64
        gend = min(hi, (g + 1) * 64)
        p0 = x - g * 64
        m = (gend - x) if p0 == 0 else min(32, gend - x)
        pieces.append((g, p0, m, x - lo))
        x += m
    return pieces


# ------------------------------------------------------------ device program

def _build_program(plan):
    import concourse.mybir as mybir
    from concourse.bacc import Bacc
    from concourse.tile import TileContext
    from concourse.alu_op_type import AluOpType

    f32 = mybir.dt.float32
    f16 = mybir.dt.float16
    u16 = mybir.dt.uint16
    i16 = mybir.dt.int16

    T = plan["T"]
    lo, hi = plan["lo"], plan["hi"]
    W = [int(hi[t] - lo[t]) for t in range(T)]
    TG = (T + 2) // 3             # tiles per partition-group of ptsA

    nc = Bacc()

    # points lhsT packed into partition groups {0..2, 32..34, 64..66}: tile t
    # lives at partitions 32*(t%3).. and free cols (t//3)*128..
    ptsA_d = nc.dram_tensor("ptsA", [67, TG * PPT], f32, kind="ExternalInput")
    cenA_d = nc.dram_tensor("cenA", [67, G], f32, kind="ExternalInput")
    ncn_d = nc.dram_tensor("ncnrep", [128, G], f32, kind="ExternalInput")
    npn_d = nc.dram_tensor("npn", [128, T], f32, kind="ExternalInput")
    sel_d = nc.dram_tensor("sel", [128, T, CELLS], f16, kind="ExternalInput")
    feat_d = nc.dram_tensor("featp", [64, 8, DIM], f16, kind="ExternalInput")
    out_d = nc.dram_tensor("out", [DIM, CELLS], f32, kind="ExternalOutput")

    ncut = ((NT - TB) // TB) * TB          # normals in full leading batches
    order = list(range(ncut)) + list(range(NT, T)) + list(range(ncut, NT))
    batches = [order[i:i + TB] for i in range(0, ncut, TB)]
    rest = order[ncut:]                    # wides + tail normals
    h1 = (len(rest) + 1) // 2
    batches.append(rest[:h1])
    batches.append(rest[h1:])

    # per-chunk last contributing tile in EMISSION order + its batch index
    chunk_last = {c: -1 for c in range(8)}
    chunk_done_batch = {c: -1 for c in range(8)}
    for bi, tiles in enumerate(batches):
        for t in tiles:
            for (c, p0, m, off) in _legal_pieces(int(lo[t]), int(hi[t])):
                chunk_last[c] = t
                chunk_done_batch[c] = bi
    # chronologically-last chunk per finals region (all regions share it)
    last_chunk = max(range(8), key=lambda c: (chunk_done_batch[c], c))

    with TileContext(nc) as tc:
        with tc.sbuf_pool(name="const", bufs=1) as cpool, \
             tc.sbuf_pool(name="big", bufs=1) as gpool, \
             tc.sbuf_pool(name="ssb", bufs=6) as spool, \
             tc.sbuf_pool(name="npc", bufs=6) as npool, \
             tc.sbuf_pool(name="sp", bufs=6) as sppool, \
             tc.sbuf_pool(name="ssbw", bufs=2) as swpool, \
             tc.sbuf_pool(name="band", bufs=3) as vpool, \
             tc.sbuf_pool(name="wt", bufs=6) as wpool, \
             tc.sbuf_pool(name="wtw", bufs=2) as wwpool, \
             tc.sbuf_pool(name="fin", bufs=1) as fpool, \
             tc.psum_pool(name="ps_s", bufs=2) as ps_s, \
             tc.psum_pool(name="ps_wt", bufs=1) as ps_wt, \
             tc.psum_pool(name="ps_o", bufs=1) as ps_o:

            cenA = cpool.tile([67, G], f32, name="cenA_sb")
            nc.sync.dma_start(out=cenA, in_=cenA_d[:])
            ncnrep = cpool.tile([128, G], f32, name="ncn_sb")
            npn = cpool.tile([128, T], f32, name="npn_sb")
            feats = cpool.tile([64, 8, DIM], f16, name="feat_sb")
            ptsA = gpool.tile([67, TG * PPT], f32, name="ptsA_sb")
            sel = gpool.tile([128, T, CELLS], f16, name="sel_sb")

            # head chunk of points, then the small consts, then the rest
            nc.sync.dma_start(out=ptsA[:, 0:5 * PPT], in_=ptsA_d[:, 0:5 * PPT])
            nc.sync.dma_start(out=ncnrep, in_=ncn_d[:])
            nc.sync.dma_start(out=npn, in_=npn_d[:])
            tb = [0, 14, 40, 70, T]
            gdone = 5
            for i in range(len(tb) - 1):
                t0, t1 = tb[i], tb[i + 1]
                g1 = TG if t1 == T else (t1 + 2) // 3
                if g1 > gdone:
                    nc.sync.dma_start(
                        out=ptsA[:, gdone * PPT:g1 * PPT],
                        in_=ptsA_d[:, gdone * PPT:g1 * PPT],
                    )
                    gdone = g1
                nc.sync.dma_start(out=sel[:, t0:t1, :], in_=sel_d[:, t0:t1, :])
                if i == 1:
                    nc.sync.dma_start(out=feats, in_=feat_d[:])

            wt_pairs = [
                ps_wt.tile([64, 512], f32, name=f"wt_ps{c}", tag=f"wt_ps{c}")
                for c in range(4)
            ]
            wt_ps = [
                wt_pairs[c // 2][:, (c % 2) * 256:(c % 2) * 256 + CELLS]
                for c in range(8)
            ]
            for c in range(4):
                nc.vector.memset(wt_pairs[c], 0)

            state = {}

            def emit_sel_batch(bi):
                tiles = batches[bi]
                nb = len(tiles)
                vband = vpool.tile([128, nb, 8], f32, name=f"vb{bi}", tag="vb")
                iband = vpool.tile([128, nb, 8], u16, name=f"ib{bi}", tag="ib")
                for j, t in enumerate(tiles):
                    w = W[t]
                    ptA = ptsA[32 * (t % 3):32 * (t % 3) + 3,
                               (t // 3) * PPT:(t // 3 + 1) * PPT]
                    if w > SLAB:
                        ssb = swpool.tile([128, G], f32, name=f"ssw{t}", tag="ssw")
                        for half in range(2):
                            s_ps = ps_s.tile(
                                [128, SLAB], f32, name=f"sp{t}_{half}", tag="sp"
                            )
                            nc.tensor.matmul(
                                out=s_ps,
                                lhsT=ptA,
                                rhs=cenA[32 * (t % 3):32 * (t % 3) + 3,
                                         half * SLAB:(half + 1) * SLAB],
                                start=True,
                                stop=True,
                            )
                            nc.scalar.copy(
                                out=ssb[:, half * SLAB:(half + 1) * SLAB], in_=s_ps
                            )
                        sscan = sppool.tile([128, G], f32, name=f"spw{t}", tag="spw")
                        npc = npool.tile([128, G], f32, name=f"npw{t}", tag="npw")
                        nc.gpsimd.tensor_scalar(
                            out=npc,
                            in0=ncnrep,
                            scalar1=npn[:, t:t + 1],
                            scalar2=None,
                            op0=AluOpType.add,
                        )
                        nc.gpsimd.tensor_tensor(
                            out=sscan, in0=ssb, in1=npc, op=AluOpType.add
                        )
                    else:
                        s_ps_full = ps_s.tile([128, SLAB], f32, name=f"sp{t}", tag="sp")
                        s_ps = s_ps_full[:, 0:w]
                        ssb_full = spool.tile([128, SLAB], f32, name=f"ss{t}", tag="ss")
                        ssb = ssb_full[:, 0:w]
                        nc.tensor.matmul(
                            out=s_ps,
                            lhsT=ptA,
                            rhs=cenA[32 * (t % 3):32 * (t % 3) + 3, lo[t]:hi[t]],
                            start=True,
                            stop=True,
                        )
                        nc.scalar.copy(out=ssb, in_=s_ps)
                        npc_full = npool.tile([128, SLAB], f32, name=f"np{t}", tag="np")
                        npc = npc_full[:, 0:w]
                        # npc = fl(-cn + -pn), matching the reference's fl(pn+cn)
                        nc.gpsimd.tensor_scalar(
                            out=npc,
                            in0=ncnrep[:, lo[t]:hi[t]],
                            scalar1=npn[:, t:t + 1],
                            scalar2=None,
                            op0=AluOpType.add,
                        )
                        sp_full = sppool.tile([128, SLAB], f32, name=f"s{t}", tag="s")
                        sscan = sp_full[:, 0:w]
                        # s = fl(2dot + -(pn+cn)) = -d2 bitwise vs reference
                        nc.gpsimd.tensor_tensor(
                            out=sscan, in0=ssb, in1=npc, op=AluOpType.add
                        )
                    nc.vector.max(out=vband[:, j, :], in_=sscan)
                    nc.vector.max_index(
                        out=iband[:, j, :], in_max=vband[:, j, :], in_values=sscan
                    )
                d2 = vpool.tile([128, nb, 3], f32, name=f"d2{bi}", tag="d2")
                nc.gpsimd.tensor_scalar(
                    out=d2,
                    in0=vband[:, :, 0:3],
                    scalar1=-1.0,
                    scalar2=1e-10,
                    op0=AluOpType.mult,
                    op1=AluOpType.max,
                )
                rec = vpool.tile([128, nb, 3], f32, name=f"rc{bi}", tag="rc")
                nc.vector.reciprocal(out=rec, in_=d2)
                rsum = vpool.tile([128, nb, 1], f32, name=f"rs{bi}", tag="rs")
                nc.vector.tensor_reduce(
                    out=rsum[:, :, 0], in_=rec, axis=mybir.AxisListType.X,
                    op=AluOpType.add,
                )
                rinv = vpool.tile([128, nb, 1], f32, name=f"ri{bi}", tag="ri")
                nc.vector.reciprocal(out=rinv, in_=rsum)
                w4 = vpool.tile([128, nb, 4], f16, name=f"w4{bi}", tag="w4")
                nc.gpsimd.memset(w4, 0)
                nc.gpsimd.tensor_tensor(
                    out=w4[:, :, 0:3],
                    in0=rec,
                    in1=rinv.broadcast_to([128, nb, 3]),
                    op=AluOpType.mult,
                )
                i4 = vpool.tile([128, nb, 4], i16, name=f"i4{bi}", tag="i4")
                nc.gpsimd.memset(i4, -1)
                nc.gpsimd.tensor_copy(
                    out=i4[:, :, 0:3], in_=iband[:, :, 0:3].bitcast(i16)
                )
                state[bi] = (w4, i4)

            def emit_scatter_batch(bi):
                tiles = batches[bi]
                w4, i4 = state.pop(bi)
                for j, t in enumerate(tiles):
                    w = W[t]
                    if w > SLAB:
                        wt = wwpool.tile([128, G], f16, name=f"wtw{t}", tag="wtw")
                        nelem = G
                    else:
                        wt_full = wpool.tile([128, SLAB], f16, name=f"wt{t}", tag="wt")
                        wt = wt_full[:, 0:w]
                        nelem = w
                    nc.gpsimd.local_scatter(
                        out_ap=wt,
                        data_ap=w4[:, j, :],
                        idxs_ap=i4[:, j, :],
                        channels=128,
                        num_elems=nelem,
                        num_idxs=4,
                    )
                    for (c, p0, m, off) in _legal_pieces(int(lo[t]), int(hi[t])):
                        nc.tensor.matmul(
                            out=wt_ps[c][p0:p0 + m, :],
                            lhsT=wt[:, off:off + m],
                            rhs=sel[:, t, :],
                            start=False,
                            stop=(t == chunk_last[c]),
                            skip_group_check=True,
                        )

            o_pair = ps_o.tile([128, 2, CELLS], f32, name="o_pair", tag="o_pair")
            o_last = ps_o.tile([128, CELLS], f32, name="o_last", tag="o_last")
            nc.scalar.memzero(o_pair)
            nc.scalar.memzero(o_last)

            def emit_chunk_finals(c):
                wsb = fpool.tile([64, CELLS], f16, name=f"wsb{c}")
                nc.scalar.copy(out=wsb, in_=wt_ps[c])
                for dc in range(3):
                    o_ps = o_pair[:, dc, :] if dc < 2 else o_last
                    nc.tensor.matmul(
                        out=o_ps,
                        lhsT=feats[:, c, dc * 128:(dc + 1) * 128],
                        rhs=wsb,
                        start=False,
                        stop=(c == last_chunk),
                        skip_group_check=True,
                    )

            prev = None
            for bi in range(len(batches)):
                emit_sel_batch(bi)
                if prev is not None:
                    emit_scatter_batch(prev)
                    for c in range(8):
                        if chunk_done_batch[c] == prev:
                            emit_chunk_finals(c)
                prev = bi
            emit_scatter_batch(prev)
            for c in range(8):
                if chunk_done_batch[c] == prev:
                    emit_chunk_finals(c)

            for dc in range(3):
                o_ps = o_pair[:, dc, :] if dc < 2 else o_last
                osb = fpool.tile([128, CELLS], f32, name=f"osb{dc}")
                nc.scalar.copy(out=osb, in_=o_ps)
                dma_eng = (nc.sync, nc.scalar, nc.gpsimd)[dc]
                dma_eng.dma_start(out=out_d[dc * 128:(dc + 1) * 128, :], in_=osb)

    nc.finalize()
    return nc


# ------------------------------------------------------------- host: inputs

def _host_inputs(plan, group_features, group_centers, original_points, core):
    b, h = core // 2, core % 2
    T = plan["T"]
    TG = (T + 2) // 3
    order, cen_s, _ = plan["batch"][b]
    src = plan["cores"][core]["src"]

    pts = np.asarray(
        original_points[b, h * HALF:(h + 1) * HALF], dtype=np.float32
    )

    psrc = np.where(src >= 0, src, 0).reshape(-1)          # (T*PPT,)
    p = pts[psrc]                                           # (T*PPT, 3) fp32
    # pn in reference add order: (x*x + y*y) + z*z, fp32
    pn = (p[:, 0] * p[:, 0] + p[:, 1] * p[:, 1]) + p[:, 2] * p[:, 2]

    ptsA = np.zeros((67, TG * PPT), dtype=np.float32)
    rows = np.repeat((np.arange(T) % 3) * 32, PPT)
    cols = np.repeat(np.arange(T) // 3, PPT) * PPT + np.tile(np.arange(PPT), T)
    for i in range(3):
        ptsA[rows + i, cols] = 2.0 * p[:, i]

    # npn[p, t] = -pn of point p in tile t
    npn = np.zeros((128, T), dtype=np.float32)
    npn[np.tile(np.arange(PPT), T), np.repeat(np.arange(T), PPT)] = -pn

    cs = cen_s.astype(np.float32)
    cn = (cs[:, 0] * cs[:, 0] + cs[:, 1] * cs[:, 1]) + cs[:, 2] * cs[:, 2]
    cenA = np.zeros((67, G), dtype=np.float32)
    for gset in range(3):
        cenA[32 * gset:32 * gset + 3] = cs.T
    ncnrep = np.ascontiguousarray(np.tile(-cn[None, :], (PPT, 1)))

    sel = np.zeros((128, T, CELLS), dtype=np.float16)
    gidx = h * HALF + psrc
    row = gidx // IMAGE
    col = gidx % IMAGE
    cell = (row // KS - 7 * h) * POOL + col // KS
    tt = np.repeat(np.arange(T), PPT)
    pp = np.tile(np.arange(PPT), T)
    valid = src.reshape(-1) >= 0
    sel[pp[valid], tt[valid], cell[valid]] = 1.0 / 64.0

    feat = np.asarray(group_features[b], dtype=np.float32)[order]
    featp = np.ascontiguousarray(
        feat.reshape(8, 64, DIM).transpose(1, 0, 2)
    ).astype(np.float16)

    return {
        "ptsA": ptsA,
        "cenA": cenA,
        "ncnrep": ncnrep,
        "npn": npn,
        "sel": sel,
        "featp": featp,
    }


# ------------------------------------------------------------------ fallback

def _numpy_fallback(group_features, group_centers, original_points,
                    nonzero_indices, kernel_size):
    gf = np.asarray(group_features, dtype=np.float64)
    cen = np.asarray(group_centers, dtype=np.float64)
    pts = np.asarray(original_points, dtype=np.float64)
    ks = int(kernel_size)
    out = np.zeros((B, DIM, IMAGE * IMAGE), dtype=np.float64)
    for b in range(B):
        d2 = (
            np.sum(pts[b] ** 2, axis=1)[:, None]
            + np.sum(cen[b] ** 2, axis=1)[None, :]
            - 2.0 * pts[b] @ cen[b].T
        )
        idx = np.argsort(d2, axis=1)[:, :3]
        d = np.maximum(np.take_along_axis(d2, idx, axis=1), 1e-10)
        rec = 1.0 / d
        w = rec / rec.sum(axis=1, keepdims=True)
        interp = np.einsum("nkd,nk->dn", gf[b][idx], w)
        out[b][:, np.asarray(nonzero_indices)] = interp
    ho = IMAGE // ks
    pooled = out.reshape(B, DIM, ho, ks, ho, ks).mean(axis=(3, 5))
    return pooled.astype(np.float32)


# -------------------------------------------------------------------- kernel

def kernel(group_features, group_centers, original_points, nonzero_indices,
           kernel_size):
    nz = np.asarray(nonzero_indices)
    ks = int(np.asarray(kernel_size))
    if ks != KS or nz.shape != (N,) or not np.array_equal(nz, np.arange(N)):
        return _numpy_fallback(
            group_features, group_centers, original_points, nonzero_indices,
            kernel_size,
        )

    from concourse.bass_utils import run_bass_kernel_spmd

    plan = _plan(group_centers, original_points)
    nc = _build_program(plan)
    _CACHE["nc"] = nc
    _CACHE["plan"] = plan

    in_maps = [
        _host_inputs(plan, group_features, group_centers, original_points, c)
        for c in range(NCORES)
    ]
    res = run_bass_kernel_spmd(nc, in_maps, core_ids=list(range(NCORES))).results

    out = np.zeros((B, DIM, POOL, POOL), dtype=np.float32)
    for c in range(NCORES):
        b, h = c // 2, c % 2
        out[b, :, 7 * h:7 * h + 7, :] = res[c]["out"].reshape(DIM, 7, POOL)
    return out
